# revision 9
# baseline (speedup 1.0000x reference)
"""Trainium2 Bass kernel for PersonalizedCalibrationNetwork (MoE-style judge routing).

Strategy: expert-parallel over the judge axis. Judge j lives on core j // 8.
The host routes samples to the core owning their judge, groups them by judge,
and pads every judge group to a uniform capacity C (so the single SPMD program
is shape-identical on all 8 cores). Each core computes, for its samples:

    z1 = sigmoid(x_aug @ (W1 + W1a[j]))      x_aug = [x, 1]
    z2 = sigmoid(z1_aug @ (W2 + W2a[j]))
    out = z2_aug @ (V + Va[j])               flattened to [257, 64]

All matmuls run transposed (features on partitions): z^T = G^T @ x^T, so layer
outputs feed the next layer without transposes. Judges are processed in
independent streams (groups of up to 4 sharing a PSUM bank strip); per group:
  - the shared weight part is a full-width matmul,
  - each judge's weight part accumulates into its column slice,
  - all bias rows (8 judge + 1 shared) are applied by ONE K=9 matmul against a
    host-built 0/1 block mask [9, N].
Streams are emitted depth-first (stream 0 runs layers 1-3 while stream 1's
weights arrive). Because each dma_start carries ~2us of ring-serial completion
latency, the host packs everything into 3 giant contiguous transfers: a "head"
blob (x^T | shared weights | biases+mask) and one all-layer weight blob per
stream. Inputs are bf16; accumulation is fp32 in PSUM.
"""

import ml_dtypes
import numpy as np

import concourse.mybir as mybir
import concourse.tile as tile
from concourse import bacc
from concourse.bass_utils import run_bass_kernel_spmd


class _SlimTileContext(tile.TileContext):
    """TileContext with a slimmer kernel tail: one all-engine barrier
    before the semaphore clears instead of two (each engine simply halts
    after the clears; NRT waits for all engines before NEFF completion)."""

    def _drain_and_barrier(self, tick_clock, wait_clock):
        drain_inst = self.nc.sync.drain()
        wait_clock.add_sem_waits(
            drain_inst.ins, tile.ScopedClock({None: tick_clock.global_clock}))
        self.nc.all_engine_barrier()
        popped = self.nc._tile_sem_poison_stack.pop()
        assert popped is self._sem_poison
        self.nc.clear_and_free_semaphores(
            list(self.sems.allocated().values()))


N_CORES = 8
J = 64                 # judges
JPC = J // N_CORES     # judges per core
IN = 256               # input features (+1 bias)
L1 = 256
L2 = 256
Q = 16
A = 4
QA = Q * A             # 64 output columns
P = 128                # partitions
PSUM_W = 512           # fp32 psum bank width
KB = JPC + 1           # bias-matmul contraction dim (8 judge rows + shared)
NB = L1 + L2 + QA      # bias pack columns
WJ = 2 * (L1 + L2 + QA)  # per-judge weight blob columns (1152)

BF16 = True
NP_W = ml_dtypes.bfloat16 if BF16 else np.float32

_cache = {}


def _make_groups(C):
    """Split the 8 judges into groups whose column strip fits a PSUM bank."""
    per_group = max(1, min(4, PSUM_W // C)) if C <= PSUM_W else 1
    groups = []  # (col0, gw, [(judge, ncol, width), ...])
    if C <= PSUM_W:
        for g0 in range(0, JPC, per_group):
            js = list(range(g0, min(g0 + per_group, JPC)))
            blocks = [(jj, jj * C, C) for jj in js]
            groups.append((g0 * C, len(js) * C, blocks))
    else:
        for jj in range(JPC):
            for c0 in range(0, C, PSUM_W):
                w = min(PSUM_W, C - c0)
                groups.append((jj * C + c0, w, [(jj, jj * C + c0, w)]))
    return groups


def _build_program(C):
    """Build + compile the SPMD Bass program for per-judge capacity C."""
    N = JPC * C  # padded samples per core
    groups = _make_groups(C)
    n_streams = len(groups)
    NH = 2 * N + 2 * NB + NB + N  # head cols: xT | wsh | bm(padded rows)

    nc = bacc.Bacc("TRN2", target_bir_lowering=False, debug=False,
                   num_devices=N_CORES)
    dt = mybir.dt.bfloat16 if BF16 else mybir.dt.float32
    f32 = mybir.dt.float32

    # head blob [P, NH]: cols [0:2N) x^T (ko-major), [2N:2N+1152) shared
    # weights (ko-major), [2N+1152:NH) bias+mask on partitions 0..8 only
    head_d = nc.dram_tensor("head", [P, NH], dt, kind="ExternalInput")
    ws_d = [nc.dram_tensor(f"ws{s}", [P, len(groups[s][2]) * WJ], dt,
                           kind="ExternalInput") for s in range(n_streams)]
    out_d = nc.dram_tensor("outT", [QA, N], f32, kind="ExternalOutput")

    X0 = 0            # x^T base col: ko*N + n
    W0 = 2 * N        # shared weights base col: ko*NB + m
    B0 = 2 * N + 2 * NB   # bias base col (partitions 0..8): + m
    M0 = B0 + NB      # mask base col (partitions 0..8): + n

    with _SlimTileContext(nc) as tc:
        with (
            tc.tile_pool(name="const", bufs=1) as const,
            tc.tile_pool(name="psum", bufs=5, space="PSUM") as psum,
        ):
            head = const.tile([P, NH], dt, tag="head")
            ws = [const.tile([P, len(groups[s][2]) * WJ], dt, tag=f"ws{s}",
                             name=f"ws{s}") for s in range(n_streams)]
            z1T = const.tile([P, 2, N], dt, tag="z1T")
            z2T = const.tile([P, 2, N], dt, tag="z2T")
            outT = const.tile([QA, N], f32, tag="outT")

            # 3 giant loads in need order on the two HWDGE rings
            nc.scalar.dma_start(head[:], head_d[:])
            nc.sync.dma_start(ws[0][:], ws_d[0][:])
            for s in range(1, n_streams):
                nc.scalar.dma_start(ws[s][:], ws_d[s][:])

            def glayer(s, li, rhs_of, M, zout):
                """One layer of stream s: z^T = act(W^T @ rhs + b).

                rhs_of(ko, c0, w) -> [128, w] rhs slice; li = layer index.
                """
                col0, gw, blocks = groups[s]
                sh_off = [0, L1, L1 + L2][li]
                wj_off = [0, 2 * L1, 2 * (L1 + L2)][li]
                n_m = (M + P - 1) // P
                for m in range(n_m):
                    mw = min(P, M - m * P)
                    ps = psum.tile([P, PSUM_W], f32, tag="ps",
                                   name="ps")[:mw, :gw]
                    for ko in range(2):
                        nc.tensor.matmul(
                            ps,
                            head[:, W0 + ko * NB + sh_off + m * P:
                                 W0 + ko * NB + sh_off + m * P + mw],
                            rhs_of(ko, col0, gw),
                            start=(ko == 0), stop=False)
                    nc.tensor.matmul(
                        ps,
                        head[:KB, B0 + sh_off + m * P:B0 + sh_off + m * P + mw],
                        head[:KB, M0 + col0:M0 + col0 + gw],
                        start=False, stop=False)
                    for bi, (jj, ncol, w) in enumerate(blocks):
                        off = ncol - col0
                        base = bi * WJ + wj_off
                        for ko in range(2):
                            nc.tensor.matmul(
                                ps[:, off:off + w],
                                ws[s][:, base + ko * mwof(li) + m * P:
                                      base + ko * mwof(li) + m * P + mw],
                                rhs_of(ko, ncol, w),
                                start=False,
                                stop=(bi == len(blocks) - 1 and ko == 1))
                    if zout is not None:
                        nc.scalar.activation(
                            zout[:mw, m, col0:col0 + gw], ps,
                            mybir.ActivationFunctionType.Sigmoid)
                    else:
                        nc.vector.tensor_copy(
                            outT[:mw, col0:col0 + gw], ps)

            def mwof(li):  # per-layer output width (cols per ko in blob)
                return [L1, L2, QA][li]

            def rhs_x(ko, c0, w):
                return head[:, X0 + ko * N + c0:X0 + ko * N + c0 + w]

            def rhs_of_tile(t):
                return lambda ko, c0, w: t[:, ko, c0:c0 + w]

            for s in range(n_streams):
                glayer(s, 0, rhs_x, L1, z1T)
                glayer(s, 1, rhs_of_tile(z1T), L2, z2T)
                glayer(s, 2, rhs_of_tile(z2T), QA, None)

            nc.sync.dma_start(out_d[:], outT[:])

    nc.compile()
    return nc, N, groups


def kernel(X_machine_evals, X_human_judges, W1, W1a, W2, W2a, V, Va):
    X = np.asarray(X_machine_evals, dtype=np.float32)
    jid = np.asarray(X_human_judges).reshape(-1).astype(np.int64)
    W1 = np.asarray(W1, dtype=np.float32)
    W1a = np.asarray(W1a, dtype=np.float32)
    W2 = np.asarray(W2, dtype=np.float32)
    W2a = np.asarray(W2a, dtype=np.float32)
    V = np.asarray(V, dtype=np.float32)
    Va = np.asarray(Va, dtype=np.float32)
    B = X.shape[0]

    counts = np.bincount(jid, minlength=J)
    C = int(counts.max())

    if C not in _cache:
        _cache[C] = _build_program(C)
    nc, N, groups = _cache[C]

    # stable order of sample indices grouped by judge
    order = np.argsort(jid, kind="stable")
    sorted_j = jid[order]

    def pack_w(w):  # [256, M] -> [128, 2*M] (ko-major cols)
        M = w.shape[1]
        return w[:256].reshape(2, P, M).transpose(1, 0, 2).reshape(P, 2 * M)

    Vf = V.transpose(1, 0, 2).reshape(IN + 1, QA)          # [257, 64]
    Vaf = Va.transpose(0, 2, 1, 3).reshape(J, IN + 1, QA)  # [J, 257, 64]

    # shared-weight block [2, P, NB] -> [P, 2*NB] ko-major
    wsh_cols = np.concatenate(
        [W1[:256].reshape(2, P, L1), W2[:256].reshape(2, P, L2),
         Vf[:256].reshape(2, P, QA)], axis=2)
    wsh_flat = wsh_cols.transpose(1, 0, 2).reshape(P, 2 * NB)

    mask_in = np.zeros((KB, N), dtype=np.float32)
    mask_in[JPC, :] = 1
    for k in range(JPC):
        mask_in[k, k * C:(k + 1) * C] = 1

    NH = 2 * N + 2 * NB + NB + N

    in_maps = []
    core_meta = []
    for c in range(N_CORES):
        judges = np.arange(c * JPC, (c + 1) * JPC)
        Xp = np.zeros((N, IN), dtype=np.float32)
        samp = []  # per-judge sample indices
        for k, jj in enumerate(judges):
            idx = order[np.searchsorted(sorted_j, jj):
                        np.searchsorted(sorted_j, jj, side="right")]
            Xp[k * C:k * C + len(idx)] = X[idx]
            samp.append(idx)
        core_meta.append(samp)

        head = np.zeros((P, NH), dtype=np.float32)
        head[:, :2 * N] = Xp.T.reshape(2, P, N).transpose(1, 0, 2).reshape(
            P, 2 * N)
        head[:, 2 * N:2 * N + 2 * NB] = wsh_flat
        b0 = 2 * N + 2 * NB
        head[:JPC, b0:b0 + L1] = W1a[judges, 256]
        head[:JPC, b0 + L1:b0 + L1 + L2] = W2a[judges, 256]
        head[:JPC, b0 + L1 + L2:b0 + NB] = Vaf[judges, 256]
        head[JPC, b0:b0 + NB] = np.concatenate([W1[256], W2[256], Vf[256]])
        head[:KB, b0 + NB:] = mask_in

        im = {"head": head.astype(NP_W)}
        for s, (_, _, blocks) in enumerate(groups):
            js = judges[[b[0] for b in blocks]]
            blob = np.concatenate(
                [np.concatenate([pack_w(W1a[jj]), pack_w(W2a[jj]),
                                 pack_w(Vaf[jj])], axis=1) for jj in js],
                axis=1)
            im[f"ws{s}"] = np.ascontiguousarray(blob.astype(NP_W))
        in_maps.append(im)

    res = run_bass_kernel_spmd(nc, in_maps, core_ids=list(range(N_CORES)))

    out = np.zeros((B, Q, A), dtype=np.float32)
    for c in range(N_CORES):
        oT = res.results[c]["outT"]          # [64, N]
        o = oT.T.reshape(N, Q, A)
        for k, idx in enumerate(core_meta[c]):
            out[idx] = o[k * C:k * C + len(idx)]
    return out
